# revision 34
# baseline (speedup 1.0000x reference)
"""Trainium2 Bass kernel for sorted-segment sum+mean (segment_reduce).

out[g] = concat(mean_g, sum_g) over rows of nbr_fea grouped by sorted
segment_ids; out shape [num_segments, 2*D].

Strategy
--------
Rows are sorted by segment id, so each segment is a contiguous row range.
Segments are packed greedily into "chunks" of at most S=24 consecutive
segments and at most T*128 rows (T chosen to minimize total padded rows,
~2.5% overhead).  Chunks are grouped 4 per "supergroup"; each supergroup's
rows are packed (on host) into 4*T row-tiles of 128 rows, laid out
[supergroup][partition][chunk][tile][feat] so each supergroup loads with
fully contiguous DMAs.

The f32 features are cast to bf16 on host (rel err ~2^-9, far inside the
2e-2 gate), halving HBM traffic versus f32 — this kernel is memory-bound
(~68.8MB/core; ~325 GB/s sustained vs the ~420 GB/s no-compute burst rate;
the gap is SBUF-fabric contention with the PE's operand reads).

On device, per 128-row tile, a one-hot matrix U[row, slot] = (rel_id == slot)
is built on the VectorEngine (is_equal against an iota constant) in fp8e4
(0/1 exact; halves LDWEIGHTS + DVE write traffic) and used as the matmul
stationary operand; the moving operand is the [128, 64] bf16 x tile.  The 4
chunks of a supergroup accumulate into four 32-partition strips of ONE
[128, 64] PSUM tile via col-tiled matmuls (tile_position=(0,32j), 128x32
array mode); loop order is t-major (j inner) so consecutive instructions hit
different col groups.

DMA discipline (the load-bearing part):
- x triggers are software-pipelined LOOKAHEAD=8 supergroups ahead of
  consumption, so both HWDGE rings always have several transfers queued.  A
  trigger emitted after a compute-coupled op would run only ~1 ahead and drop
  that ring into ~150 GB/s latency-bound mode.
- each supergroup loads as TWO column-half transfers: 5376B descriptor lines
  ship as 26.2 GB/s/port packets, where whole 10752B lines drop to 19.
- sync/scalar rings alternate by supergroup; the epilogue (ACT mean via
  per-partition 1/count scale + sum copy) never precedes a trigger in either
  ring's stream; out flushes ride gpsimd (SWDGE), final flush on sync.

Padding rows carry rel_id = -1 so their one-hot row is all zero; unused
slots produce zeros the host discards.  The kernel is compiled AFTER seeing
the inputs, so the (data-dependent) chunk plan is a compile-time constant;
one SPMD program runs on all 8 cores.
"""

import ml_dtypes
import numpy as np

import concourse.bass as bass
import concourse.mybir as mybir
import concourse.tile as tile
from concourse import bass_utils

N_TOTAL = 4_194_304
D = 64                       # feature dim
G = 32_768                   # num segments
N_CORES = 8
S = 24                       # segment slots per chunk (one PSUM col-tile strip)
JJ = 4                       # chunks per supergroup (4 x 32 = 128 psum partitions)
P = 128                      # rows per tile == SBUF partitions

F32 = mybir.dt.float32
BF16 = mybir.dt.bfloat16
FP8 = mybir.dt.float8e4
I8 = mybir.dt.int8
NP_BF16 = ml_dtypes.bfloat16


def _split_syncs(nc, max_waits=1):
    """This container's walrus accepts at most one sync-wait per instruction;
    split extra waits onto preceding same-engine NoOps (engine stalls at each
    wait in turn, so semantics are identical)."""
    n_split = 0
    for f in nc.m.functions:
        for bb in f.blocks:
            new_insts = []
            for ins in bb.instructions:
                si = getattr(ins, "sync_info", None)
                waits = list(si.on_wait) if si is not None and si.on_wait else []
                if len(waits) > max_waits:
                    n_split += 1
                    extra = waits[:-max_waits]
                    for i in range(0, len(extra), max_waits):
                        nop = mybir.InstNoOp(
                            name=f"{ins.name}_wsplit{i}", ins=[], outs=[]
                        )
                        nop.engine = ins.engine
                        nop.sync_info = mybir.SyncInfo(
                            on_wait=extra[i : i + max_waits], on_update=[]
                        )
                        new_insts.append(nop)
                    si.on_wait = waits[-max_waits:]
                new_insts.append(ins)
            bb.instructions = new_insts
    return n_split


def _build_bass(T, SG, split_syncs=True):
    """Build the SPMD program: SG supergroups per core, JJ chunks each,
    T row-tiles per chunk."""
    nc = bass.Bass("TRN2", debug=False, num_devices=1)

    JT = JJ * T  # tiles per supergroup
    x_d = nc.dram_tensor("x", [SG, P, JT * D], BF16, kind="ExternalInput")
    rel_d = nc.dram_tensor("rel", [P, SG * JT], I8, kind="ExternalInput")
    iota_d = nc.dram_tensor("iota", [P, JJ * T * S], I8, kind="ExternalInput")
    recip_d = nc.dram_tensor("recip", [P, SG], F32, kind="ExternalInput")
    out_d = nc.dram_tensor("out", [P, SG * 2 * D], F32, kind="ExternalOutput")

    flush_every = -(-SG // 16)  # ceil: stage output DMA in ~16ths

    with tile.TileContext(nc) as tc:
        with (
            tc.tile_pool(name="const", bufs=1) as const_pool,
            tc.tile_pool(name="xin", bufs=10) as x_pool,
            tc.tile_pool(name="oh", bufs=6) as oh_pool,
            tc.tile_pool(name="outs", bufs=2) as out_pool,
            tc.tile_pool(name="ps", bufs=8, space="PSUM") as ps_pool,
        ):
            # head order matters: iota + the first rel quarter land before
            # the first x supergroup so burst 0 starts ~14us in, not ~17us
            iota_sb = const_pool.tile([P, JJ * T * S], I8)
            nc.sync.dma_start(iota_sb[:], iota_d[:])
            recip_sb = const_pool.tile([P, SG], F32)
            nc.scalar.dma_start(recip_sb[:], recip_d[:])
            rel_sb = const_pool.tile([P, SG * JT], I8)
            rq = (-(-SG // 4)) * JT  # rel quarter (whole supergroups)
            nc.sync.dma_start(rel_sb[:, :rq], rel_d[:, :rq])
            nc.scalar.dma_start(rel_sb[:, rq : 2 * rq], rel_d[:, rq : 2 * rq])

            flushed = 0
            out_sb = None
            xts = {}
            LOOKAHEAD = 8  # trigger x(sg+LOOKAHEAD) before this sg's
            # epilogue so both HWDGE rings always run ~LOOKAHEAD transfers
            # ahead of consumption (a ring trigger emitted after ACT would
            # be held to ~1 ahead and fall into ~150 GB/s latency-bound mode)
            xh = (JT * D) // 2

            def trigger(k, split=False):
                xt = x_pool.tile([P, JT * D], BF16)
                xts[k] = xt
                # two column-half transfers: forces 5376B descriptor lines
                # (26.2 GB/s/port packets; whole 10752B lines drop to 19)
                if split:  # head: halves ride both rings in parallel
                    nc.sync.dma_start(xt[:, :xh], x_d[k, :, :xh])
                    nc.scalar.dma_start(xt[:, xh:], x_d[k, :, xh:])
                    return
                eng = nc.sync if k % 2 == 0 else nc.scalar
                eng.dma_start(xt[:, :xh], x_d[k, :, :xh])
                eng.dma_start(xt[:, xh:], x_d[k, :, xh:])

            for sg in range(SG):
                if sg == 0:
                    trigger(0, split=True)
                    trigger(1, split=True)
                    nc.sync.dma_start(
                        rel_sb[:, 2 * rq : 3 * rq], rel_d[:, 2 * rq : 3 * rq]
                    )
                    nc.scalar.dma_start(
                        rel_sb[:, 3 * rq :], rel_d[:, 3 * rq :]
                    )
                    for k in range(2, min(LOOKAHEAD, SG)):
                        trigger(k)
                if sg + LOOKAHEAD < SG:
                    trigger(sg + LOOKAHEAD)
                xt = xts.pop(sg)

                oh = oh_pool.tile([P, JT * S], FP8)
                nc.vector.tensor_tensor(
                    oh[:],
                    rel_sb[:, sg * JT : (sg + 1) * JT].to_broadcast((P, JT, S)),
                    iota_sb[:],
                    mybir.AluOpType.is_equal,
                )
                ps = ps_pool.tile([P, D], F32)
                # j-blocked pairs: the j<2 matmuls only touch xt's first
                # column half, so they start as soon as that transfer lands
                # (the second half arrives while they run)
                for jp in range(JJ // 2):
                    for t in range(T):
                        for j in (2 * jp, 2 * jp + 1):
                            k = j * T + t
                            nc.tensor.matmul(
                                ps[32 * j : 32 * j + S, :],
                                oh[:, k * S : (k + 1) * S],
                                xt[:, k * D : (k + 1) * D],
                                start=(t == 0),
                                stop=(t == T - 1),
                                tile_position=(0, 32 * j),
                            )
                if out_sb is None:
                    out_sb = out_pool.tile([P, flush_every * 2 * D], F32)
                base = (sg - flushed) * 2 * D
                # mean = sum * (1/count)   (per-partition scale, on ACT)
                nc.scalar.activation(
                    out_sb[:, base : base + D],
                    ps[:],
                    mybir.ActivationFunctionType.Copy,
                    scale=recip_sb[:, sg : sg + 1],
                )
                nc.scalar.copy(out_sb[:, base + D : base + 2 * D], ps[:])
                if sg + 1 == SG or (sg + 1) % flush_every == 0 or sg >= SG - 3:
                    q0 = flushed * 2 * D
                    q1 = (sg + 1) * 2 * D
                    eng = nc.sync if sg + 1 == SG else nc.gpsimd
                    eng.dma_start(out_d[:, q0:q1], out_sb[:, 0 : q1 - q0])
                    flushed = sg + 1
                    out_sb = None

    if split_syncs:
        _split_syncs(nc)
    return nc


def _greedy_plan(counts):
    """Pack consecutive segments into chunks with <=S segments and <=T*128
    rows, scanning candidate capacities T to minimize total padded rows.
    Returns (T, bases, nsegs) arrays (unpadded chunk list)."""
    g_total = len(counts)
    t_min = max(1, int(-(-int(counts.max()) // P)))
    # aim near S segments per chunk
    t_avg = max(t_min, -(-int(counts.sum()) * S // (g_total * P)))
    best = None
    for T in range(max(t_min, t_avg - 6), max(t_min, t_avg) + 3):
        cap = T * P
        bases, nsegs = [], []
        g = 0
        r = 0
        n = 0
        while g + n < g_total:
            cnt = counts[g + n]
            if n < S and r + cnt <= cap:
                r += cnt
                n += 1
            else:
                assert n > 0, "single segment exceeds chunk capacity"
                bases.append(g)
                nsegs.append(n)
                g += n
                r = 0
                n = 0
        if n > 0:
            bases.append(g)
            nsegs.append(n)
        ct = len(bases)
        c_per = -(-ct // (N_CORES * JJ)) * JJ  # chunks/core, whole supergroups
        total = c_per * N_CORES * cap
        if best is None or total < best[0]:
            best = (total, T, np.array(bases), np.array(nsegs))
    _, T, bases, nsegs = best
    return T, bases, nsegs


def _plan_and_pack(x, seg):
    """Host-side: greedy chunk plan + packed/padded device arrays."""
    x = np.ascontiguousarray(x, dtype=np.float32)
    seg = np.asarray(seg).astype(np.int64)

    counts = np.bincount(seg, minlength=G).astype(np.int64)
    seg_row_start = np.zeros(G + 1, dtype=np.int64)
    np.cumsum(counts, out=seg_row_start[1:])
    recip = (1.0 / np.maximum(counts, 1.0)).astype(np.float32)

    T, bases, nsegs = _greedy_plan(counts)
    C = -(-len(bases) // (N_CORES * JJ)) * JJ  # chunks per core
    SG = C // JJ  # supergroups per core
    ct_pad = C * N_CORES
    pad = ct_pad - len(bases)
    # empty padding chunks (0 segments, 0 rows)
    bases_p = np.concatenate([bases, np.zeros(pad, dtype=np.int64)])
    nsegs_p = np.concatenate([nsegs, np.zeros(pad, dtype=np.int64)])
    row_start = seg_row_start[bases_p]
    n_rows = seg_row_start[bases_p + nsegs_p] - row_start

    # row index for [chunk, partition, tile]: row = start_c + t*128 + p
    ridx = (
        row_start[:, None, None]
        + np.arange(P, dtype=np.int64)[None, :, None]
        + (np.arange(T, dtype=np.int64) * P)[None, None, :]
    )
    valid = ridx < (row_start + n_rows)[:, None, None]
    ridx_c = np.where(valid, ridx, 0)

    # regroup so each supergroup of JJ chunks has contiguous per-partition
    # lines: [nsg_total, P, JJ, T, D]
    NSG = ct_pad // JJ
    ridx_b = ridx_c.reshape(NSG, JJ, P, T).transpose(0, 2, 1, 3)
    valid_b = valid.reshape(NSG, JJ, P, T).transpose(0, 2, 1, 3)
    xg = x[ridx_b.reshape(-1)].reshape(NSG, P, JJ, T, D)
    xg[~valid_b] = 0.0
    xbuf = xg.astype(NP_BF16).reshape(NSG, P, JJ * T * D)
    del xg

    rel = seg[ridx_c] - bases_p[:, None, None]
    relbuf = np.where(valid, rel, -1).astype(np.int8)  # [ct_pad, P, T]

    iota_np = np.tile(
        np.arange(S, dtype=np.int8), (P, JJ * T)
    )

    # per-slot reciprocal: psum partition 32*j+s of supergroup sg ->
    # segment bases[core*C + sg*JJ + j] + s (1.0 pad)
    gidx = bases_p[:, None] + np.arange(S, dtype=np.int64)[None, :]
    slot_valid = np.arange(S)[None, :] < nsegs_p[:, None]
    recip_slots = np.where(
        slot_valid, recip[np.clip(gidx, 0, G - 1)], np.float32(1.0)
    ).astype(np.float32)  # [ct_pad, S]

    in_maps = []
    for core in range(N_CORES):
        c0, c1 = core * C, (core + 1) * C
        # rel columns: (sg, j, t) -> col (sg*JJ + j)*T + t  == chunk-major
        rel_core = relbuf[c0:c1].transpose(1, 0, 2).reshape(P, C * T)
        # recip partitions: p = 32*j + s (strips are 32-aligned), free dim sg
        rc = np.ones((P, SG), np.float32)
        rc.reshape(JJ, 32, SG)[:, :S, :] = (
            recip_slots[c0:c1].reshape(SG, JJ, S).transpose(1, 2, 0)
        )
        in_maps.append(
            {
                "x": np.ascontiguousarray(xbuf[core * SG : (core + 1) * SG]),
                "rel": np.ascontiguousarray(rel_core),
                "iota": iota_np,
                "recip": np.ascontiguousarray(rc),
            }
        )
    plan = dict(T=T, SG=SG, C=C, gidx=gidx, slot_valid=slot_valid)
    return plan, in_maps


def _assemble(results, plan):
    """[core]["out"] of shape [128, SG*2*D] -> [G, 2*D] via slot->segment."""
    SG = plan["SG"]
    # [128, SG, 2, D] -> partition p = 32*j + s (strips are 32-aligned)
    vs = [
        results[core]["out"].reshape(JJ, 32, SG, 2, D)[:, :S]
        for core in range(N_CORES)
    ]
    # chunk index within core: c = sg*JJ + j -> order (sg, j)
    mean = np.concatenate(
        [v[:, :, :, 0, :].transpose(2, 0, 1, 3).reshape(SG * JJ, S, D) for v in vs]
    )  # [ct_pad, S, D]
    ssum = np.concatenate(
        [v[:, :, :, 1, :].transpose(2, 0, 1, 3).reshape(SG * JJ, S, D) for v in vs]
    )
    out = np.empty((G, 2 * D), np.float32)
    m = plan["slot_valid"]
    out[plan["gidx"][m], :D] = mean[m]
    out[plan["gidx"][m], D:] = ssum[m]
    return out


def _run_impl(nbr_fea, segment_ids, num_segments, trace=False, trace_kwargs=None):
    assert int(num_segments) == G, f"expected {G} segments, got {num_segments}"
    assert nbr_fea.shape == (N_TOTAL, D), nbr_fea.shape

    plan, in_maps = _plan_and_pack(nbr_fea, segment_ids)
    nc = _build_bass(plan["T"], plan["SG"])
    kw = {}
    if trace:
        kw = dict(trace=True, **(trace_kwargs or {}))
    res = bass_utils.run_bass_kernel_spmd(
        nc, in_maps, core_ids=list(range(N_CORES)), **kw
    )
    return _assemble(res.results, plan), res


def kernel(nbr_fea, segment_ids, num_segments):
    out, _ = _run_impl(np.asarray(nbr_fea), np.asarray(segment_ids), num_segments)
    return out


# revision 35
# speedup vs baseline: 1.0108x; 1.0108x over previous
"""Trainium2 Bass kernel for sorted-segment sum+mean (segment_reduce).

out[g] = concat(mean_g, sum_g) over rows of nbr_fea grouped by sorted
segment_ids; out shape [num_segments, 2*D].

Strategy
--------
Rows are sorted by segment id, so each segment is a contiguous row range.
Segments are packed greedily into "chunks" of at most S=24 consecutive
segments and at most T*128 rows (T chosen to minimize total padded rows,
~2.5% overhead).  Chunks are grouped 4 per "supergroup"; each supergroup's
rows are packed (on host) into 4*T row-tiles of 128 rows, laid out
[supergroup][partition][chunk][tile][feat] so each supergroup loads with
fully contiguous DMAs.

The f32 features are cast to bf16 on host (rel err ~2^-9, far inside the
2e-2 gate), halving HBM traffic versus f32 — this kernel is memory-bound
(~68.8MB/core; ~325 GB/s sustained vs the ~420 GB/s no-compute burst rate;
the gap is SBUF-fabric contention with the PE's operand reads).

On device, per 128-row tile, a one-hot matrix U[row, slot] = (rel_id == slot)
is built on the VectorEngine (is_equal against an iota constant) in fp8e4
(0/1 exact; halves LDWEIGHTS + DVE write traffic) and used as the matmul
stationary operand; the moving operand is the [128, 64] bf16 x tile.  The 4
chunks of a supergroup accumulate into four 32-partition strips of ONE
[128, 64] PSUM tile via col-tiled matmuls (tile_position=(0,32j), 128x32
array mode); loop order is t-major (j inner) so consecutive instructions hit
different col groups.

DMA discipline (the load-bearing part):
- x triggers are software-pipelined LOOKAHEAD=8 supergroups ahead of
  consumption, so both HWDGE rings always have several transfers queued.  A
  trigger emitted after a compute-coupled op would run only ~1 ahead and drop
  that ring into ~150 GB/s latency-bound mode.
- each supergroup loads as TWO column-half transfers: 5376B descriptor lines
  ship as 26.2 GB/s/port packets, where whole 10752B lines drop to 19.
- sync/scalar rings alternate by supergroup; the epilogue (ACT mean via
  per-partition 1/count scale + sum copy) never precedes a trigger in either
  ring's stream; out flushes ride gpsimd (SWDGE), final flush on sync.

Padding rows carry rel_id = -1 so their one-hot row is all zero; unused
slots produce zeros the host discards.  The kernel is compiled AFTER seeing
the inputs, so the (data-dependent) chunk plan is a compile-time constant;
one SPMD program runs on all 8 cores.
"""

import ml_dtypes
import numpy as np

import concourse.bass as bass
import concourse.mybir as mybir
import concourse.tile as tile
from concourse import bass_utils

N_TOTAL = 4_194_304
D = 64                       # feature dim
G = 32_768                   # num segments
N_CORES = 8
S = 24                       # segment slots per chunk (one PSUM col-tile strip)
JJ = 4                       # chunks per supergroup (4 x 32 = 128 psum partitions)
P = 128                      # rows per tile == SBUF partitions

F32 = mybir.dt.float32
BF16 = mybir.dt.bfloat16
FP8 = mybir.dt.float8e4
NP_BF16 = ml_dtypes.bfloat16


def _split_syncs(nc, max_waits=1):
    """This container's walrus accepts at most one sync-wait per instruction;
    split extra waits onto preceding same-engine NoOps (engine stalls at each
    wait in turn, so semantics are identical)."""
    n_split = 0
    for f in nc.m.functions:
        for bb in f.blocks:
            new_insts = []
            for ins in bb.instructions:
                si = getattr(ins, "sync_info", None)
                waits = list(si.on_wait) if si is not None and si.on_wait else []
                if len(waits) > max_waits:
                    n_split += 1
                    extra = waits[:-max_waits]
                    for i in range(0, len(extra), max_waits):
                        nop = mybir.InstNoOp(
                            name=f"{ins.name}_wsplit{i}", ins=[], outs=[]
                        )
                        nop.engine = ins.engine
                        nop.sync_info = mybir.SyncInfo(
                            on_wait=extra[i : i + max_waits], on_update=[]
                        )
                        new_insts.append(nop)
                    si.on_wait = waits[-max_waits:]
                new_insts.append(ins)
            bb.instructions = new_insts
    return n_split


def _build_bass(T, SG, split_syncs=True):
    """Build the SPMD program: SG supergroups per core, JJ chunks each,
    T row-tiles per chunk."""
    nc = bass.Bass("TRN2", debug=False, num_devices=1)

    JT = JJ * T  # tiles per supergroup
    x_d = nc.dram_tensor("x", [SG, P, JT * D], BF16, kind="ExternalInput")
    rel_d = nc.dram_tensor("rel", [P, SG * JT], BF16, kind="ExternalInput")
    iota_d = nc.dram_tensor("iota", [P, JJ * T * S], BF16, kind="ExternalInput")
    recip_d = nc.dram_tensor("recip", [P, SG], F32, kind="ExternalInput")
    out_d = nc.dram_tensor("out", [P, SG * 2 * D], F32, kind="ExternalOutput")

    flush_every = -(-SG // 16)  # ceil: stage output DMA in ~16ths

    with tile.TileContext(nc) as tc:
        with (
            tc.tile_pool(name="const", bufs=1) as const_pool,
            tc.tile_pool(name="xin", bufs=10) as x_pool,
            tc.tile_pool(name="oh", bufs=6) as oh_pool,
            tc.tile_pool(name="outs", bufs=2) as out_pool,
            tc.tile_pool(name="ps", bufs=8, space="PSUM") as ps_pool,
        ):
            # head order matters: iota + the first rel quarter land before
            # the first x supergroup so burst 0 starts ~14us in, not ~17us
            iota_sb = const_pool.tile([P, JJ * T * S], BF16)
            nc.sync.dma_start(iota_sb[:], iota_d[:])
            recip_sb = const_pool.tile([P, SG], F32)
            nc.scalar.dma_start(recip_sb[:], recip_d[:])
            rel_sb = const_pool.tile([P, SG * JT], BF16)
            rq = (-(-SG // 4)) * JT  # rel quarter (whole supergroups)
            nc.sync.dma_start(rel_sb[:, :rq], rel_d[:, :rq])
            nc.scalar.dma_start(rel_sb[:, rq : 2 * rq], rel_d[:, rq : 2 * rq])

            flushed = 0
            out_sb = None
            xts = {}
            LOOKAHEAD = 8  # trigger x(sg+LOOKAHEAD) before this sg's
            # epilogue so both HWDGE rings always run ~LOOKAHEAD transfers
            # ahead of consumption (a ring trigger emitted after ACT would
            # be held to ~1 ahead and fall into ~150 GB/s latency-bound mode)
            xh = (JT * D) // 2

            def trigger(k, split=False):
                xt = x_pool.tile([P, JT * D], BF16)
                xts[k] = xt
                # two column-half transfers: forces 5376B descriptor lines
                # (26.2 GB/s/port packets; whole 10752B lines drop to 19)
                if split:  # head: halves ride both rings in parallel
                    nc.sync.dma_start(xt[:, :xh], x_d[k, :, :xh])
                    nc.scalar.dma_start(xt[:, xh:], x_d[k, :, xh:])
                    return
                eng = nc.sync if k % 2 == 0 else nc.scalar
                eng.dma_start(xt[:, :xh], x_d[k, :, :xh])
                eng.dma_start(xt[:, xh:], x_d[k, :, xh:])

            for sg in range(SG):
                if sg == 0:
                    trigger(0, split=True)
                    trigger(1, split=True)
                    nc.sync.dma_start(
                        rel_sb[:, 2 * rq : 3 * rq], rel_d[:, 2 * rq : 3 * rq]
                    )
                    nc.scalar.dma_start(
                        rel_sb[:, 3 * rq :], rel_d[:, 3 * rq :]
                    )
                    for k in range(2, min(LOOKAHEAD, SG)):
                        trigger(k)
                if sg + LOOKAHEAD < SG:
                    trigger(sg + LOOKAHEAD)
                xt = xts.pop(sg)

                oh = oh_pool.tile([P, JT * S], FP8)
                nc.vector.tensor_tensor(
                    oh[:],
                    rel_sb[:, sg * JT : (sg + 1) * JT].to_broadcast((P, JT, S)),
                    iota_sb[:],
                    mybir.AluOpType.is_equal,
                )
                ps = ps_pool.tile([P, D], F32)
                for t in range(T):
                    for j in range(JJ):
                        k = j * T + t
                        nc.tensor.matmul(
                            ps[32 * j : 32 * j + S, :],
                            oh[:, k * S : (k + 1) * S],
                            xt[:, k * D : (k + 1) * D],
                            start=(t == 0),
                            stop=(t == T - 1),
                            tile_position=(0, 32 * j),
                        )
                if out_sb is None:
                    out_sb = out_pool.tile([P, flush_every * 2 * D], F32)
                base = (sg - flushed) * 2 * D
                # mean = sum * (1/count)   (per-partition scale, on ACT)
                nc.scalar.activation(
                    out_sb[:, base : base + D],
                    ps[:],
                    mybir.ActivationFunctionType.Copy,
                    scale=recip_sb[:, sg : sg + 1],
                )
                nc.scalar.copy(out_sb[:, base + D : base + 2 * D], ps[:])
                if sg + 1 == SG or (sg + 1) % flush_every == 0 or sg >= SG - 3:
                    q0 = flushed * 2 * D
                    q1 = (sg + 1) * 2 * D
                    eng = nc.sync if sg + 1 == SG else nc.gpsimd
                    eng.dma_start(out_d[:, q0:q1], out_sb[:, 0 : q1 - q0])
                    flushed = sg + 1
                    out_sb = None

    if split_syncs:
        _split_syncs(nc)
    return nc


def _greedy_plan(counts):
    """Pack consecutive segments into chunks with <=S segments and <=T*128
    rows, scanning candidate capacities T to minimize total padded rows.
    Returns (T, bases, nsegs) arrays (unpadded chunk list)."""
    g_total = len(counts)
    t_min = max(1, int(-(-int(counts.max()) // P)))
    # aim near S segments per chunk
    t_avg = max(t_min, -(-int(counts.sum()) * S // (g_total * P)))
    best = None
    for T in range(max(t_min, t_avg - 6), max(t_min, t_avg) + 3):
        cap = T * P
        bases, nsegs = [], []
        g = 0
        r = 0
        n = 0
        while g + n < g_total:
            cnt = counts[g + n]
            if n < S and r + cnt <= cap:
                r += cnt
                n += 1
            else:
                assert n > 0, "single segment exceeds chunk capacity"
                bases.append(g)
                nsegs.append(n)
                g += n
                r = 0
                n = 0
        if n > 0:
            bases.append(g)
            nsegs.append(n)
        ct = len(bases)
        c_per = -(-ct // (N_CORES * JJ)) * JJ  # chunks/core, whole supergroups
        total = c_per * N_CORES * cap
        if best is None or total < best[0]:
            best = (total, T, np.array(bases), np.array(nsegs))
    _, T, bases, nsegs = best
    return T, bases, nsegs


def _plan_and_pack(x, seg):
    """Host-side: greedy chunk plan + packed/padded device arrays."""
    x = np.ascontiguousarray(x, dtype=np.float32)
    seg = np.asarray(seg).astype(np.int64)

    counts = np.bincount(seg, minlength=G).astype(np.int64)
    seg_row_start = np.zeros(G + 1, dtype=np.int64)
    np.cumsum(counts, out=seg_row_start[1:])
    recip = (1.0 / np.maximum(counts, 1.0)).astype(np.float32)

    T, bases, nsegs = _greedy_plan(counts)
    C = -(-len(bases) // (N_CORES * JJ)) * JJ  # chunks per core
    SG = C // JJ  # supergroups per core
    ct_pad = C * N_CORES
    pad = ct_pad - len(bases)
    # empty padding chunks (0 segments, 0 rows)
    bases_p = np.concatenate([bases, np.zeros(pad, dtype=np.int64)])
    nsegs_p = np.concatenate([nsegs, np.zeros(pad, dtype=np.int64)])
    row_start = seg_row_start[bases_p]
    n_rows = seg_row_start[bases_p + nsegs_p] - row_start

    # row index for [chunk, partition, tile]: row = start_c + t*128 + p
    ridx = (
        row_start[:, None, None]
        + np.arange(P, dtype=np.int64)[None, :, None]
        + (np.arange(T, dtype=np.int64) * P)[None, None, :]
    )
    valid = ridx < (row_start + n_rows)[:, None, None]
    ridx_c = np.where(valid, ridx, 0)

    # regroup so each supergroup of JJ chunks has contiguous per-partition
    # lines: [nsg_total, P, JJ, T, D]
    NSG = ct_pad // JJ
    ridx_b = ridx_c.reshape(NSG, JJ, P, T).transpose(0, 2, 1, 3)
    valid_b = valid.reshape(NSG, JJ, P, T).transpose(0, 2, 1, 3)
    xg = x[ridx_b.reshape(-1)].reshape(NSG, P, JJ, T, D)
    xg[~valid_b] = 0.0
    xbuf = xg.astype(NP_BF16).reshape(NSG, P, JJ * T * D)
    del xg

    rel = seg[ridx_c] - bases_p[:, None, None]
    relbuf = np.where(valid, rel, -1).astype(NP_BF16)  # [ct_pad, P, T]

    iota_np = np.tile(
        np.arange(S, dtype=np.float32), (P, JJ * T)
    ).astype(NP_BF16)

    # per-slot reciprocal: psum partition 32*j+s of supergroup sg ->
    # segment bases[core*C + sg*JJ + j] + s (1.0 pad)
    gidx = bases_p[:, None] + np.arange(S, dtype=np.int64)[None, :]
    slot_valid = np.arange(S)[None, :] < nsegs_p[:, None]
    recip_slots = np.where(
        slot_valid, recip[np.clip(gidx, 0, G - 1)], np.float32(1.0)
    ).astype(np.float32)  # [ct_pad, S]

    in_maps = []
    for core in range(N_CORES):
        c0, c1 = core * C, (core + 1) * C
        # rel columns: (sg, j, t) -> col (sg*JJ + j)*T + t  == chunk-major
        rel_core = relbuf[c0:c1].transpose(1, 0, 2).reshape(P, C * T)
        # recip partitions: p = 32*j + s (strips are 32-aligned), free dim sg
        rc = np.ones((P, SG), np.float32)
        rc.reshape(JJ, 32, SG)[:, :S, :] = (
            recip_slots[c0:c1].reshape(SG, JJ, S).transpose(1, 2, 0)
        )
        in_maps.append(
            {
                "x": np.ascontiguousarray(xbuf[core * SG : (core + 1) * SG]),
                "rel": np.ascontiguousarray(rel_core),
                "iota": iota_np,
                "recip": np.ascontiguousarray(rc),
            }
        )
    plan = dict(T=T, SG=SG, C=C, gidx=gidx, slot_valid=slot_valid)
    return plan, in_maps


def _assemble(results, plan):
    """[core]["out"] of shape [128, SG*2*D] -> [G, 2*D] via slot->segment."""
    SG = plan["SG"]
    # [128, SG, 2, D] -> partition p = 32*j + s (strips are 32-aligned)
    vs = [
        results[core]["out"].reshape(JJ, 32, SG, 2, D)[:, :S]
        for core in range(N_CORES)
    ]
    # chunk index within core: c = sg*JJ + j -> order (sg, j)
    mean = np.concatenate(
        [v[:, :, :, 0, :].transpose(2, 0, 1, 3).reshape(SG * JJ, S, D) for v in vs]
    )  # [ct_pad, S, D]
    ssum = np.concatenate(
        [v[:, :, :, 1, :].transpose(2, 0, 1, 3).reshape(SG * JJ, S, D) for v in vs]
    )
    out = np.empty((G, 2 * D), np.float32)
    m = plan["slot_valid"]
    out[plan["gidx"][m], :D] = mean[m]
    out[plan["gidx"][m], D:] = ssum[m]
    return out


def _run_impl(nbr_fea, segment_ids, num_segments, trace=False, trace_kwargs=None):
    assert int(num_segments) == G, f"expected {G} segments, got {num_segments}"
    assert nbr_fea.shape == (N_TOTAL, D), nbr_fea.shape

    plan, in_maps = _plan_and_pack(nbr_fea, segment_ids)
    nc = _build_bass(plan["T"], plan["SG"])
    kw = {}
    if trace:
        kw = dict(trace=True, **(trace_kwargs or {}))
    res = bass_utils.run_bass_kernel_spmd(
        nc, in_maps, core_ids=list(range(N_CORES)), **kw
    )
    return _assemble(res.results, plan), res


def kernel(nbr_fea, segment_ids, num_segments):
    out, _ = _run_impl(np.asarray(nbr_fea), np.asarray(segment_ids), num_segments)
    return out
